# revision 4
# baseline (speedup 1.0000x reference)
"""Multi-head attention + residual + LayerNorm TRN2 Bass kernel.

Problem: B=8, S=1024, d_model=512, 16 heads x d_k=128.
Returns (out, attn) like the reference:
    out  (8, 1024, 512)  f32   layernorm(context @ W_fc + input_Q)
    attn (8, 16, 1024, 1024) f32  softmax probabilities

Sharding: data-parallel over batch, one batch element per NeuronCore (8 cores).

Per-core design (all matmuls fp16 in / f32 PSUM accumulate):
  - host pre-transposes X_q/X_k/X_v to (512,1024) fp16; scales W_Q by 1/sqrt(128)
  - V = X_v @ W_V computed once in (seq, 2048) layout
  - per head h:
      qT_h/kT_h (128,1024) via W as stationary, X^T as moving
      pass B (q-rows on partitions): S = Q K^T tiles -> exp (ACT, no max-trick;
        scores are O(1)) -> mask-mul with row-sum accumulation in one DVE op ->
        reciprocal -> normalize -> attn tile out (f32)
      pass A (k-rows on partitions): S^T = K Q^T tiles -> exp -> mask-mul ->
        context matmuls C^T_h = V_h^T @ E^T accumulated in PSUM; scaled by the
        row-reciprocals (broadcast across partitions via a small DRAM
        round-trip) and stashed to DRAM as fp16
  - fc: O = C^T.T @ W_fc accumulated over all 16 head blocks + residual,
    then LayerNorm via bn_stats/bn_aggr + sqrt + reciprocal.
"""

import os
import sys

for _p in ("/opt/trn_rl_repo", "/root/.axon_site/_ro/trn_rl_repo"):
    if os.path.isdir(_p) and _p not in sys.path:
        sys.path.append(_p)

import numpy as np

import concourse.bass as bass
import concourse.bacc as bacc
import concourse.tile as tile
import concourse.mybir as mybir
from concourse import bass_utils

F32 = mybir.dt.float32
F16 = mybir.dt.float16
AF = mybir.ActivationFunctionType
ALU = mybir.AluOpType

S = 1024
DM = 512
H = 16
DK = 128
NQT = S // 128  # 8 q/k tiles of 128
EPS = 1e-5


def build_kernel(n_cores: int = 8):
    nc = bacc.Bacc(
        "TRN2",
        target_bir_lowering=False,
        debug=False,
        enable_asserts=False,
        num_devices=n_cores,
    )

    # ---- DRAM I/O (per core) ----
    xqT = nc.dram_tensor("xqT", (DM, S), F16, kind="ExternalInput")
    xkT = nc.dram_tensor("xkT", (DM, S), F16, kind="ExternalInput")
    xvT = nc.dram_tensor("xvT", (DM, S), F16, kind="ExternalInput")
    xq = nc.dram_tensor("xq", (S, DM), F32, kind="ExternalInput")
    wq = nc.dram_tensor("wq", (DM, H * DK), F16, kind="ExternalInput")
    wk = nc.dram_tensor("wk", (DM, H * DK), F16, kind="ExternalInput")
    wv = nc.dram_tensor("wv", (DM, H * DK), F16, kind="ExternalInput")
    wfc = nc.dram_tensor("wfc", (H * DK, DM), F16, kind="ExternalInput")
    m01 = nc.dram_tensor("m01", (S, S), F16, kind="ExternalInput")
    m01T = nc.dram_tensor("m01T", (S, S), F16, kind="ExternalInput")

    attn = nc.dram_tensor("attn", (H, S, S), F32, kind="ExternalOutput")
    outp = nc.dram_tensor("outp", (S, DM), F32, kind="ExternalOutput")

    with tile.TileContext(nc) as tc:
        _body(nc, tc, xqT, xkT, xvT, xq, wq, wk, wv, wfc, m01, m01T, attn, outp)

    nc.compile()
    return nc


def _body(nc, tc, xqT, xkT, xvT, xq, wq, wk, wv, wfc, m01, m01T, attn, outp):
    from concourse.masks import make_identity
    from contextlib import ExitStack

    with ExitStack() as ctx:
        consts = ctx.enter_context(tc.tile_pool(name="consts", bufs=1))
        dram = ctx.enter_context(tc.tile_pool(name="dram", bufs=1, space="DRAM"))

        # ---- resident SBUF tensors ----
        wq_sb = [consts.tile([128, H * DK], F16, name=f"wq{t}") for t in range(4)]
        wk_sb = [consts.tile([128, H * DK], F16, name=f"wk{t}") for t in range(4)]
        xqT_sb = [consts.tile([128, S], F16, name=f"xqT{t}") for t in range(4)]
        xkT_sb = [consts.tile([128, S], F16, name=f"xkT{t}") for t in range(4)]
        m01_sb = [consts.tile([128, S], F16, name=f"m01_{t}") for t in range(NQT)]
        m01T_sb = [consts.tile([128, S], F16, name=f"m01T_{t}") for t in range(NQT)]
        v_sb = [consts.tile([128, H * DK], F16, name=f"v{t}") for t in range(NQT)]
        ident = consts.tile([128, 128], F32)
        make_identity(nc, ident)

        for t in range(4):
            nc.sync.dma_start(wq_sb[t], wq.ap()[t * 128 : (t + 1) * 128, :])
            nc.sync.dma_start(wk_sb[t], wk.ap()[t * 128 : (t + 1) * 128, :])
            nc.sync.dma_start(xqT_sb[t], xqT.ap()[t * 128 : (t + 1) * 128, :])
            nc.sync.dma_start(xkT_sb[t], xkT.ap()[t * 128 : (t + 1) * 128, :])
        for t in range(NQT):
            nc.sync.dma_start(m01_sb[t], m01.ap()[t * 128 : (t + 1) * 128, :])
            nc.sync.dma_start(m01T_sb[t], m01T.ap()[t * 128 : (t + 1) * 128, :])

        # DRAM scratch
        craw_d = dram.tile([H, 128, S], F16)
        recipT_d = dram.tile([H, S], F32)

        # ---- V projection: V (seq, H*DK), computed once ----
        with (
            tc.tile_pool(name="p0", bufs=1) as p0,
            tc.tile_pool(name="p0ps", bufs=2, space="PSUM") as p0ps,
        ):
            xvT_sb = [p0.tile([128, S], F16, name=f"xvT{t}") for t in range(4)]
            wv_sb = [p0.tile([128, H * DK], F16, name=f"wv{t}") for t in range(4)]
            for t in range(4):
                nc.sync.dma_start(xvT_sb[t], xvT.ap()[t * 128 : (t + 1) * 128, :])
                nc.sync.dma_start(wv_sb[t], wv.ap()[t * 128 : (t + 1) * 128, :])
            for st in range(NQT):
                vps = p0ps.tile([128, H * DK], F32, tag="vps")
                for nch in range(4):
                    for kt in range(4):
                        nc.tensor.matmul(
                            vps[:, nch * 512 : (nch + 1) * 512],
                            xvT_sb[kt][:, st * 128 : (st + 1) * 128],
                            wv_sb[kt][:, nch * 512 : (nch + 1) * 512],
                            start=(kt == 0),
                            stop=(kt == 3),
                        )
                nc.vector.tensor_copy(v_sb[st], vps)

        # ---- head loop ----
        with (
            tc.tile_pool(name="hd", bufs=2) as hd,
            tc.tile_pool(name="work", bufs=3) as work,
            tc.tile_pool(name="scps", bufs=2, space="PSUM") as scps,
            tc.tile_pool(name="crps", bufs=2, space="PSUM") as crps,
        ):
            for h in range(H):
                hsl = slice(h * DK, (h + 1) * DK)

                # -- projections for this head: qT_h/kT_h (128, S)
                qT_h = hd.tile([128, S], F16, name="qT_h")
                kT_h = hd.tile([128, S], F16, name="kT_h")
                for dst, w_sb, x_sb in ((qT_h, wq_sb, xqT_sb), (kT_h, wk_sb, xkT_sb)):
                    pps = scps.tile([128, S], F32, tag="sc")
                    for qc in range(2):
                        for kt in range(4):
                            nc.tensor.matmul(
                                pps[:, qc * 512 : (qc + 1) * 512],
                                w_sb[kt][:, hsl],
                                x_sb[kt][:, qc * 512 : (qc + 1) * 512],
                                start=(kt == 0),
                                stop=(kt == 3),
                            )
                    nc.vector.tensor_copy(dst, pps)

                # -- pass B: q-major softmax + attn output
                recips = hd.tile([128, NQT], F32, name="recips")
                for qt in range(NQT):
                    sps = scps.tile([128, S], F32, tag="sc")
                    for kc in range(2):
                        nc.tensor.matmul(
                            sps[:, kc * 512 : (kc + 1) * 512],
                            qT_h[:, qt * 128 : (qt + 1) * 128],
                            kT_h[:, kc * 512 : (kc + 1) * 512],
                            start=True,
                            stop=True,
                        )
                    e_t = work.tile([128, S], F16, name="e_t")
                    nc.scalar.activation(e_t, sps, AF.Exp)
                    em_t = work.tile([128, S], F16, name="em_t")
                    nc.vector.scalar_tensor_tensor(
                        out=em_t,
                        in0=e_t,
                        scalar=1.0,
                        in1=m01_sb[qt],
                        op0=ALU.mult,
                        op1=ALU.mult,
                        accum_out=recips[:, qt : qt + 1],
                    )
                    nc.vector.reciprocal(
                        recips[:, qt : qt + 1], recips[:, qt : qt + 1]
                    )
                    p_t = work.tile([128, S], F32, name="p_t")
                    nc.vector.tensor_scalar(
                        out=p_t,
                        in0=em_t,
                        scalar1=recips[:, qt : qt + 1],
                        scalar2=None,
                        op0=ALU.mult,
                    )
                    nc.sync.dma_start(
                        attn.ap()[h, qt * 128 : (qt + 1) * 128, :], p_t
                    )

                # transpose recips (128, 8) -> (8, 128) -> DRAM row (1024,)
                tps = crps.tile([NQT, 128], F32, tag="cr")
                nc.tensor.transpose(tps, recips, ident)
                recipT_row = work.tile([NQT, 128], F32, name="recipT_row")
                nc.vector.tensor_copy(recipT_row, tps)
                nc.sync.dma_start(
                    recipT_d[h].rearrange("(a b) -> a b", a=NQT), recipT_row
                )

                # broadcast-read the reciprocals across partitions (128, S)
                rrow = recipT_d[h : h + 1, :]
                recipT_bc = work.tile([128, S], F32, name="recipT_bc")
                nc.sync.dma_start(
                    recipT_bc,
                    bass.AP(tensor=rrow.tensor, offset=rrow.offset, ap=[[0, 128], [1, S]]),
                )

                # -- pass A: k-major E^T and context matmuls
                cps = crps.tile([128, S], F32, tag="cr")
                for kt in range(NQT):
                    stps = scps.tile([128, S], F32, tag="sc")
                    for qc in range(2):
                        nc.tensor.matmul(
                            stps[:, qc * 512 : (qc + 1) * 512],
                            kT_h[:, kt * 128 : (kt + 1) * 128],
                            qT_h[:, qc * 512 : (qc + 1) * 512],
                            start=True,
                            stop=True,
                        )
                    et_t = work.tile([128, S], F16, name="et_t")
                    nc.scalar.activation(et_t, stps, AF.Exp)
                    etm_t = work.tile([128, S], F16, name="etm_t")
                    nc.gpsimd.tensor_tensor(
                        out=etm_t, in0=et_t, in1=m01T_sb[kt], op=ALU.mult
                    )
                    for qc in range(2):
                        nc.tensor.matmul(
                            cps[:, qc * 512 : (qc + 1) * 512],
                            v_sb[kt][:, hsl],
                            etm_t[:, qc * 512 : (qc + 1) * 512],
                            start=(kt == 0),
                            stop=(kt == NQT - 1),
                        )

                # scale context rows by 1/rowsum and stash to DRAM
                craws_t = work.tile([128, S], F16, name="craws_t")
                nc.vector.tensor_mul(craws_t, cps, recipT_bc)
                nc.sync.dma_start(craw_d[h], craws_t)

        # ---- fc + residual + layernorm ----
        with (
            tc.tile_pool(name="fc", bufs=1) as fc,
            tc.tile_pool(name="fcw", bufs=3) as fcw,
            tc.tile_pool(name="fcps", bufs=1, space="PSUM") as fcps,
        ):
            wfc_sb = [fc.tile([128, DM], F16, name=f"wfc{t}") for t in range(H)]
            for t in range(H):
                nc.sync.dma_start(wfc_sb[t], wfc.ap()[t * 128 : (t + 1) * 128, :])
            eps_t = fc.tile([128, 1], F32)
            nc.vector.memset(eps_t, EPS)

            ops = [
                fcps.tile([128, DM], F32, name=f"ops{qt}", tag=f"o{qt}")
                for qt in range(NQT)
            ]
            for kt2 in range(H):
                craw_t = fcw.tile([128, S], F16, name="craw_t")
                nc.sync.dma_start(craw_t, craw_d[kt2])
                for qt in range(NQT):
                    nc.tensor.matmul(
                        ops[qt],
                        craw_t[:, qt * 128 : (qt + 1) * 128],
                        wfc_sb[kt2],
                        start=(kt2 == 0),
                        stop=(kt2 == H - 1),
                    )
            for qt in range(NQT):
                xq_t = fcw.tile([128, DM], F32, name="xq_t")
                nc.sync.dma_start(xq_t, xq.ap()[qt * 128 : (qt + 1) * 128, :])
                o1 = fcw.tile([128, DM], F32, name="o1")
                nc.vector.tensor_add(o1, ops[qt], xq_t)
                stats = fcw.tile([128, 6], F32, name="stats")
                nc.vector.bn_stats(stats, o1)
                mv = fcw.tile([128, 2], F32, name="mv")
                nc.vector.bn_aggr(mv, stats)
                std = fcw.tile([128, 1], F32, name="std")
                nc.scalar.activation(
                    std, mv[:, 1:2], AF.Sqrt, bias=eps_t, scale=1.0
                )
                nc.vector.reciprocal(std, std)
                out_t = fcw.tile([128, DM], F32, name="out_t")
                nc.vector.tensor_scalar(
                    out=out_t,
                    in0=o1,
                    scalar1=mv[:, 0:1],
                    scalar2=std,
                    op0=ALU.subtract,
                    op1=ALU.mult,
                )
                nc.sync.dma_start(outp.ap()[qt * 128 : (qt + 1) * 128, :], out_t)


_CACHED = {}


def _get_kernel():
    if "nc" not in _CACHED:
        _CACHED["nc"] = build_kernel(8)
    return _CACHED["nc"]


def _prep_core_inputs(b, input_Q, input_K, input_V, attn_mask, W_Q, W_K, W_V, W_fc):
    f16 = np.float16
    scale = np.float32(1.0 / np.sqrt(DK))
    m01 = (~attn_mask[b]).astype(f16)
    return {
        "xqT": np.ascontiguousarray(input_Q[b].T).astype(f16),
        "xkT": np.ascontiguousarray(input_K[b].T).astype(f16),
        "xvT": np.ascontiguousarray(input_V[b].T).astype(f16),
        "xq": np.ascontiguousarray(input_Q[b]).astype(np.float32),
        "wq": (W_Q * scale).astype(f16),
        "wk": W_K.astype(f16),
        "wv": W_V.astype(f16),
        "wfc": W_fc.astype(f16),
        "m01": m01,
        "m01T": np.ascontiguousarray(m01.T),
    }


def kernel(input_Q, input_K, input_V, attn_mask, W_Q, W_K, W_V, W_fc, _trace=False):
    input_Q = np.asarray(input_Q, dtype=np.float32)
    input_K = np.asarray(input_K, dtype=np.float32)
    input_V = np.asarray(input_V, dtype=np.float32)
    attn_mask = np.asarray(attn_mask, dtype=bool)
    W_Q = np.asarray(W_Q, dtype=np.float32)
    W_K = np.asarray(W_K, dtype=np.float32)
    W_V = np.asarray(W_V, dtype=np.float32)
    W_fc = np.asarray(W_fc, dtype=np.float32)

    B = input_Q.shape[0]
    assert B == 8

    nc = _get_kernel()
    in_maps = [
        _prep_core_inputs(b, input_Q, input_K, input_V, attn_mask, W_Q, W_K, W_V, W_fc)
        for b in range(B)
    ]
    res = bass_utils.run_bass_kernel_spmd(
        nc, in_maps, core_ids=list(range(B)), trace=_trace
    )
    out = np.stack([res.results[b]["outp"] for b in range(B)])
    attn = np.stack([res.results[b]["attn"] for b in range(B)])
    if _trace:
        _CACHED["last_result"] = res
    return out, attn


# revision 8
# speedup vs baseline: 1.2966x; 1.2966x over previous
"""Multi-head attention + residual + LayerNorm TRN2 Bass kernel.

Problem: B=8, S=1024, d_model=512, 16 heads x d_k=128.
Returns (out, attn) like the reference:
    out  (8, 1024, 512)  f32   layernorm(context @ W_fc + input_Q)
    attn (8, 16, 1024, 1024) f32  softmax probabilities

Sharding: data-parallel over batch, one batch element per NeuronCore (8 cores).

Per-core design (all matmuls fp16 in / f32 PSUM accumulate):
  - host pre-transposes X_q/X_k/X_v to (512,1024) fp16; scales W_Q by 1/sqrt(128)
  - V = X_v @ W_V computed once in (seq, 2048) layout
  - per head h:
      qT_h/kT_h (128,1024) via W as stationary, X^T as moving
      pass B (q-rows on partitions): S = Q K^T tiles -> exp (ACT, no max-trick;
        scores are O(1)) -> mask-mul with row-sum accumulation in one DVE op ->
        reciprocal -> normalize -> attn tile out (f32)
      pass A (k-rows on partitions): S^T = K Q^T tiles -> exp -> mask-mul ->
        context matmuls C^T_h = V_h^T @ E^T accumulated in PSUM; scaled by the
        row-reciprocals (broadcast across partitions via a small DRAM
        round-trip) and stashed to DRAM as fp16
  - fc: O = C^T.T @ W_fc accumulated over all 16 head blocks + residual,
    then LayerNorm via bn_stats/bn_aggr + sqrt + reciprocal.
"""

import os
import sys

for _p in ("/opt/trn_rl_repo", "/root/.axon_site/_ro/trn_rl_repo"):
    if os.path.isdir(_p) and _p not in sys.path:
        sys.path.append(_p)

import numpy as np

import concourse.bass as bass
import concourse.bacc as bacc
import concourse.tile as tile
import concourse.mybir as mybir
from concourse import bass_utils

F32 = mybir.dt.float32
F16 = mybir.dt.float16
AF = mybir.ActivationFunctionType
ALU = mybir.AluOpType

S = 1024
DM = 512
H = 16
DK = 128
NQT = S // 128  # 8 q/k tiles of 128
EPS = 1e-5


def build_kernel(n_cores: int = 8):
    nc = bacc.Bacc(
        "TRN2",
        target_bir_lowering=False,
        debug=False,
        enable_asserts=False,
        num_devices=n_cores,
    )

    # ---- DRAM I/O (per core) ----
    xqT = nc.dram_tensor("xqT", (DM, S), F16, kind="ExternalInput")
    xkT = nc.dram_tensor("xkT", (DM, S), F16, kind="ExternalInput")
    xvT = nc.dram_tensor("xvT", (DM, S), F16, kind="ExternalInput")
    xq = nc.dram_tensor("xq", (S, DM), F32, kind="ExternalInput")
    wq = nc.dram_tensor("wq", (DM, H * DK), F16, kind="ExternalInput")
    wk = nc.dram_tensor("wk", (DM, H * DK), F16, kind="ExternalInput")
    wv = nc.dram_tensor("wv", (DM, H * DK), F16, kind="ExternalInput")
    wfc = nc.dram_tensor("wfc", (H * DK, DM), F16, kind="ExternalInput")
    m01 = nc.dram_tensor("m01", (S, S), F16, kind="ExternalInput")
    m01T = nc.dram_tensor("m01T", (S, S), F16, kind="ExternalInput")

    attn = nc.dram_tensor("attn", (H, S, S), F16, kind="ExternalOutput")
    outp = nc.dram_tensor("outp", (S, DM), F32, kind="ExternalOutput")

    with tile.TileContext(nc) as tc:
        _body(nc, tc, xqT, xkT, xvT, xq, wq, wk, wv, wfc, m01, m01T, attn, outp)

    nc.compile()
    return nc


def _body(nc, tc, xqT, xkT, xvT, xq, wq, wk, wv, wfc, m01, m01T, attn, outp):
    from concourse.masks import make_identity
    from contextlib import ExitStack

    with ExitStack() as ctx:
        consts = ctx.enter_context(tc.tile_pool(name="consts", bufs=1))
        dram = ctx.enter_context(tc.tile_pool(name="dram", bufs=1, space="DRAM"))

        # ---- resident SBUF tensors ----
        wq_sb = [consts.tile([128, H * DK], F16, name=f"wq{t}") for t in range(4)]
        wk_sb = [consts.tile([128, H * DK], F16, name=f"wk{t}") for t in range(4)]
        xqT_sb = [consts.tile([128, S], F16, name=f"xqT{t}") for t in range(4)]
        xkT_sb = [consts.tile([128, S], F16, name=f"xkT{t}") for t in range(4)]
        m01_sb = [consts.tile([128, S], F16, name=f"m01_{t}") for t in range(NQT)]
        m01T_sb = [consts.tile([128, S], F16, name=f"m01T_{t}") for t in range(NQT)]
        v_sb = [consts.tile([128, H * DK], F16, name=f"v{t}") for t in range(NQT)]
        ident = consts.tile([128, 128], F32)
        make_identity(nc, ident)

        for t in range(4):
            nc.sync.dma_start(wq_sb[t], wq.ap()[t * 128 : (t + 1) * 128, :])
            nc.sync.dma_start(wk_sb[t], wk.ap()[t * 128 : (t + 1) * 128, :])
            nc.sync.dma_start(xqT_sb[t], xqT.ap()[t * 128 : (t + 1) * 128, :])
            nc.sync.dma_start(xkT_sb[t], xkT.ap()[t * 128 : (t + 1) * 128, :])
        for t in range(NQT):
            nc.sync.dma_start(m01_sb[t], m01.ap()[t * 128 : (t + 1) * 128, :])
            nc.sync.dma_start(m01T_sb[t], m01T.ap()[t * 128 : (t + 1) * 128, :])

        # DRAM scratch
        craw_d = dram.tile([H, 128, S], F16)
        recipT_d = dram.tile([H, S], F32)

        # ---- V projection: V (seq, H*DK), computed once ----
        with (
            tc.tile_pool(name="p0", bufs=1) as p0,
            tc.tile_pool(name="p0ps", bufs=2, space="PSUM") as p0ps,
        ):
            xvT_sb = [p0.tile([128, S], F16, name=f"xvT{t}") for t in range(4)]
            wv_sb = [p0.tile([128, H * DK], F16, name=f"wv{t}") for t in range(4)]
            for t in range(4):
                nc.sync.dma_start(xvT_sb[t], xvT.ap()[t * 128 : (t + 1) * 128, :])
                nc.sync.dma_start(wv_sb[t], wv.ap()[t * 128 : (t + 1) * 128, :])
            for st in range(NQT):
                vps = p0ps.tile([128, H * DK], F32, tag="vps")
                for nch in range(4):
                    for kt in range(4):
                        nc.tensor.matmul(
                            vps[:, nch * 512 : (nch + 1) * 512],
                            xvT_sb[kt][:, st * 128 : (st + 1) * 128],
                            wv_sb[kt][:, nch * 512 : (nch + 1) * 512],
                            start=(kt == 0),
                            stop=(kt == 3),
                        )
                nc.vector.tensor_copy(v_sb[st], vps)

        # ---- head loop ----
        with (
            tc.tile_pool(name="hd", bufs=2) as hd,
            tc.tile_pool(name="work", bufs=3) as work,
            tc.tile_pool(name="scps", bufs=2, space="PSUM") as scps,
            tc.tile_pool(name="crps", bufs=2, space="PSUM") as crps,
        ):
            for h in range(H):
                hsl = slice(h * DK, (h + 1) * DK)

                # -- projections for this head: qT_h/kT_h (128, S)
                qT_h = hd.tile([128, S], F16, name="qT_h")
                kT_h = hd.tile([128, S], F16, name="kT_h")
                for dst, w_sb, x_sb in ((qT_h, wq_sb, xqT_sb), (kT_h, wk_sb, xkT_sb)):
                    pps = scps.tile([128, S], F32, tag="sc")
                    for qc in range(2):
                        for kt in range(4):
                            nc.tensor.matmul(
                                pps[:, qc * 512 : (qc + 1) * 512],
                                w_sb[kt][:, hsl],
                                x_sb[kt][:, qc * 512 : (qc + 1) * 512],
                                start=(kt == 0),
                                stop=(kt == 3),
                            )
                    nc.vector.tensor_copy(dst, pps)

                # -- pass B: q-major softmax + attn output
                recips = hd.tile([128, NQT], F32, name="recips")
                for qt in range(NQT):
                    sps = scps.tile([128, S], F32, tag="sc")
                    for kc in range(2):
                        nc.tensor.matmul(
                            sps[:, kc * 512 : (kc + 1) * 512],
                            qT_h[:, qt * 128 : (qt + 1) * 128],
                            kT_h[:, kc * 512 : (kc + 1) * 512],
                            start=True,
                            stop=True,
                        )
                    e_t = work.tile([128, S], F16, name="e_t")
                    nc.scalar.activation(e_t, sps, AF.Exp)
                    em_t = work.tile([128, S], F16, name="em_t")
                    nc.vector.scalar_tensor_tensor(
                        out=em_t,
                        in0=e_t,
                        scalar=1.0,
                        in1=m01_sb[qt],
                        op0=ALU.mult,
                        op1=ALU.mult,
                        accum_out=recips[:, qt : qt + 1],
                    )
                    nc.vector.reciprocal(
                        recips[:, qt : qt + 1], recips[:, qt : qt + 1]
                    )
                    p_t = work.tile([128, S], F16, name="p_t")
                    nc.vector.tensor_scalar(
                        out=p_t,
                        in0=em_t,
                        scalar1=recips[:, qt : qt + 1],
                        scalar2=None,
                        op0=ALU.mult,
                    )
                    nc.sync.dma_start(
                        attn.ap()[h, qt * 128 : (qt + 1) * 128, :], p_t
                    )

                # transpose recips (128, 8) -> (8, 128) -> DRAM row (1024,)
                tps = crps.tile([NQT, 128], F32, tag="cr")
                nc.tensor.transpose(tps, recips, ident)
                recipT_row = work.tile([NQT, 128], F32, name="recipT_row")
                nc.vector.tensor_copy(recipT_row, tps)
                nc.sync.dma_start(
                    recipT_d[h].rearrange("(a b) -> a b", a=NQT), recipT_row
                )

                # broadcast-read the reciprocals across partitions (128, S)
                rrow = recipT_d[h : h + 1, :]
                recipT_bc = work.tile([128, S], F32, name="recipT_bc")
                nc.sync.dma_start(
                    recipT_bc,
                    bass.AP(tensor=rrow.tensor, offset=rrow.offset, ap=[[0, 128], [1, S]]),
                )

                # -- pass A: k-major E^T and context matmuls
                cps = crps.tile([128, S], F32, tag="cr")
                for kt in range(NQT):
                    stps = scps.tile([128, S], F32, tag="sc")
                    for qc in range(2):
                        nc.tensor.matmul(
                            stps[:, qc * 512 : (qc + 1) * 512],
                            kT_h[:, kt * 128 : (kt + 1) * 128],
                            qT_h[:, qc * 512 : (qc + 1) * 512],
                            start=True,
                            stop=True,
                        )
                    et_t = work.tile([128, S], F16, name="et_t")
                    nc.scalar.activation(et_t, stps, AF.Exp)
                    etm_t = work.tile([128, S], F16, name="etm_t")
                    nc.vector.tensor_mul(etm_t, et_t, m01T_sb[kt])
                    for qc in range(2):
                        nc.tensor.matmul(
                            cps[:, qc * 512 : (qc + 1) * 512],
                            v_sb[kt][:, hsl],
                            etm_t[:, qc * 512 : (qc + 1) * 512],
                            start=(kt == 0),
                            stop=(kt == NQT - 1),
                        )

                # scale context rows by 1/rowsum and stash to DRAM
                craws_t = work.tile([128, S], F16, name="craws_t")
                nc.vector.tensor_mul(craws_t, cps, recipT_bc)
                nc.sync.dma_start(craw_d[h], craws_t)

        # ---- fc + residual + layernorm ----
        with (
            tc.tile_pool(name="fc", bufs=1) as fc,
            tc.tile_pool(name="fcw", bufs=3) as fcw,
            tc.tile_pool(name="fcps", bufs=1, space="PSUM") as fcps,
        ):
            wfc_sb = [fc.tile([128, DM], F16, name=f"wfc{t}") for t in range(H)]
            for t in range(H):
                nc.sync.dma_start(wfc_sb[t], wfc.ap()[t * 128 : (t + 1) * 128, :])
            eps_t = fc.tile([128, 1], F32)
            nc.vector.memset(eps_t, EPS)

            ops = [
                fcps.tile([128, DM], F32, name=f"ops{qt}", tag=f"o{qt}")
                for qt in range(NQT)
            ]
            for kt2 in range(H):
                craw_t = fcw.tile([128, S], F16, name="craw_t")
                nc.sync.dma_start(craw_t, craw_d[kt2])
                for qt in range(NQT):
                    nc.tensor.matmul(
                        ops[qt],
                        craw_t[:, qt * 128 : (qt + 1) * 128],
                        wfc_sb[kt2],
                        start=(kt2 == 0),
                        stop=(kt2 == H - 1),
                    )
            for qt in range(NQT):
                xq_t = fcw.tile([128, DM], F32, name="xq_t")
                nc.sync.dma_start(xq_t, xq.ap()[qt * 128 : (qt + 1) * 128, :])
                o1 = fcw.tile([128, DM], F32, name="o1")
                nc.vector.tensor_add(o1, ops[qt], xq_t)
                stats = fcw.tile([128, 6], F32, name="stats")
                nc.vector.bn_stats(stats, o1)
                mv = fcw.tile([128, 2], F32, name="mv")
                nc.vector.bn_aggr(mv, stats)
                std = fcw.tile([128, 1], F32, name="std")
                nc.scalar.activation(
                    std, mv[:, 1:2], AF.Sqrt, bias=eps_t, scale=1.0
                )
                nc.vector.reciprocal(std, std)
                out_t = fcw.tile([128, DM], F32, name="out_t")
                nc.vector.tensor_scalar(
                    out=out_t,
                    in0=o1,
                    scalar1=mv[:, 0:1],
                    scalar2=std,
                    op0=ALU.subtract,
                    op1=ALU.mult,
                )
                nc.sync.dma_start(outp.ap()[qt * 128 : (qt + 1) * 128, :], out_t)


_CACHED = {}


def _get_kernel():
    if "nc" not in _CACHED:
        _CACHED["nc"] = build_kernel(8)
    return _CACHED["nc"]


def _prep_core_inputs(b, input_Q, input_K, input_V, attn_mask, W_Q, W_K, W_V, W_fc):
    f16 = np.float16
    scale = np.float32(1.0 / np.sqrt(DK))
    m01 = (~attn_mask[b]).astype(f16)
    return {
        "xqT": np.ascontiguousarray(input_Q[b].T).astype(f16),
        "xkT": np.ascontiguousarray(input_K[b].T).astype(f16),
        "xvT": np.ascontiguousarray(input_V[b].T).astype(f16),
        "xq": np.ascontiguousarray(input_Q[b]).astype(np.float32),
        "wq": (W_Q * scale).astype(f16),
        "wk": W_K.astype(f16),
        "wv": W_V.astype(f16),
        "wfc": W_fc.astype(f16),
        "m01": m01,
        "m01T": np.ascontiguousarray(m01.T),
    }


def kernel(input_Q, input_K, input_V, attn_mask, W_Q, W_K, W_V, W_fc, _trace=False):
    input_Q = np.asarray(input_Q, dtype=np.float32)
    input_K = np.asarray(input_K, dtype=np.float32)
    input_V = np.asarray(input_V, dtype=np.float32)
    attn_mask = np.asarray(attn_mask, dtype=bool)
    W_Q = np.asarray(W_Q, dtype=np.float32)
    W_K = np.asarray(W_K, dtype=np.float32)
    W_V = np.asarray(W_V, dtype=np.float32)
    W_fc = np.asarray(W_fc, dtype=np.float32)

    B = input_Q.shape[0]
    assert B == 8

    nc = _get_kernel()
    in_maps = [
        _prep_core_inputs(b, input_Q, input_K, input_V, attn_mask, W_Q, W_K, W_V, W_fc)
        for b in range(B)
    ]
    res = bass_utils.run_bass_kernel_spmd(
        nc, in_maps, core_ids=list(range(B)), trace=_trace
    )
    out = np.stack([res.results[b]["outp"] for b in range(B)])
    attn = np.stack(
        [res.results[b]["attn"].astype(np.float32) for b in range(B)]
    )
    if _trace:
        _CACHED["last_result"] = res
    return out, attn
